# revision 4
# baseline (speedup 1.0000x reference)
"""DiscreteContinuousConv2d (sparse gnn-style conv) Trainium2 kernel.

Math: y[b,o,n] = bias[o] + sum_e psi[e] * qw[in_e] * sum_c W[o,c,k_e] * x[b, c, in_e]
      (edges e with out_e == n).

Strategy (8 NeuronCores, output sharded — no collectives):
  - Each core owns 2048 output points = 16 blocks of 128.
  - Host sorts edges by (core, block, k); pads each (block, k) group to a
    multiple of 128 ("tiles"), identical tile counts across cores (SPMD).
  - Host performs the per-edge gather (sparse im2col): G[slot, :] is the
    256-wide x row of the slot's in-point, pre-scaled by psi' =
    psi * qw[in] * 2^P (P keeps bf16 in range; y is unscaled on the host).
    G streams to the device as a dense (128, T*256) bf16 tile per block —
    the device never does random-access DMA.
  - With psi folded into G, the scatter matrices S[e, t, n] =
    (out_loc[slot] == n) are exact {0,1} one-hots — streamed as fp8
    (1.0 is exact) at half the bf16 footprint.
  - Per block on device:
      * scatter-add as matmul: z_pair[(row-half), (sub, half, n)] +=
        G_half.T @ S_tile, accumulated in PSUM (one bank per k-pair).
      * W contraction: y^T_half += BW_k.T @ z_k^T with BW_k the
        block-diagonal (over the 2 b's of a half) W_k^T. Accumulates in PSUM.
  - Host reassembles y from the per-core (block, p, n) outputs.
"""

import numpy as np
import ml_dtypes

import bass_rust
import concourse.bass as bass
from concourse import mybir
from concourse.bass_utils import run_bass_kernel_spmd
from concourse.library_overlay import lower_extended_insts
from concourse.tile import TileContext

B, CIN, COUT, K = 4, 64, 64, 9
N_IN = N_OUT = 16384
NCORES = 8
PPC = N_OUT // NCORES          # output points per core (2048)
NBLK = PPC // 128              # blocks per core (16)
ROW = B * CIN                  # gathered row width (256)

S_FP8 = True                   # S one-hot as fp8 (bf16 fallback)
_S_NP = ml_dtypes.float8_e4m3 if S_FP8 else ml_dtypes.bfloat16
_S_MY = mybir.dt.float8e4 if S_FP8 else mybir.dt.bfloat16
_G_NP = ml_dtypes.bfloat16
_G_MY = mybir.dt.bfloat16


def _prepare(x, psi_idx, psi_vals, quadrature_weights, weight):
    """Host-side sharding/sorting/layout. Returns per-core input maps."""
    xrow = np.ascontiguousarray(x.transpose(2, 0, 1).reshape(N_IN, ROW))

    k_idx = psi_idx[0].astype(np.int64)
    out_idx = psi_idx[1].astype(np.int64)
    in_idx = psi_idx[2].astype(np.int64)

    # fold quadrature into psi; rescale into dtype-friendly range (y is
    # divided by 2^P on the host afterwards)
    psi_q = psi_vals.astype(np.float64) * quadrature_weights.astype(np.float64)[in_idx]
    P = int(-np.ceil(np.log2(np.abs(psi_q).max())))
    psi_q = (psi_q * (2.0 ** P)).astype(np.float32)

    core = out_idx // PPC
    blk = (out_idx % PPC) // 128
    loc = out_idx % 128
    gid = (core * NBLK + blk) * K + k_idx          # group id, (core, blk, k)

    order = np.argsort(gid, kind="stable")
    gid_s = gid[order]
    in_s = in_idx[order]
    loc_s = loc[order]
    psi_s = psi_q[order]

    counts = np.bincount(gid_s, minlength=NCORES * NBLK * K).reshape(NCORES, NBLK, K)
    # tiles per (blk, k): shared across cores -> max
    T_bk = -(-counts.max(axis=0) // 128)           # (NBLK, K) ceil
    T_blk = T_bk.sum(axis=1)                       # (NBLK,)
    blk_base = np.concatenate([[0], np.cumsum(T_blk * 128)])  # slot offset per blk
    SLOTS = int(blk_base[-1])                      # total slots per core
    TT = SLOTS // 128

    # slot offset of each (blk, k) group
    k_base = np.zeros((NBLK, K), np.int64)
    for b in range(NBLK):
        k_base[b] = blk_base[b] + np.concatenate([[0], np.cumsum(T_bk[b] * 128)[:-1]])

    # destination slot for every (sorted) edge
    grp_start = np.zeros(NCORES * NBLK * K + 1, np.int64)
    np.cumsum(counts.reshape(-1), out=grp_start[1:])
    rank = np.arange(len(gid_s)) - grp_start[gid_s]
    g_core = gid_s // (NBLK * K)
    g_blk = (gid_s // K) % NBLK
    g_k = gid_s % K
    slot = k_base[g_blk, g_k] + rank               # slot within the core's stream

    in_maps = []
    for c in range(NCORES):
        m = g_core == c
        sl = slot[m]
        in_full = np.zeros(SLOTS, np.int64)
        in_full[sl] = in_s[m]
        psi_full = np.zeros(SLOTS, np.float32)
        psi_full[sl] = psi_s[m]
        # G: psi-scaled x rows, (SLOTS, 256) -> per blk (128, T*256)
        Gf = (xrow[in_full] * psi_full[:, None]).astype(_G_NP)
        Gd = np.empty((128, TT * 256), _G_NP)
        # S: {0,1} one-hot scatter tiles, (128, TT*128)
        Sf = np.zeros((SLOTS, 128), _S_NP)
        Sf[sl, loc_s[m]] = 1.0
        Sd = np.empty((128, TT * 128), _S_NP)
        for b in range(NBLK):
            s0, T = int(blk_base[b]), int(T_blk[b])
            t0 = s0 // 128
            Gd[:, t0 * 256:(t0 + T) * 256] = (
                Gf[s0:s0 + T * 128].reshape(T, 128, 256)
                .transpose(1, 0, 2).reshape(128, T * 256))
            Sd[:, t0 * 128:(t0 + T) * 128] = (
                Sf[s0:s0 + T * 128].reshape(T, 128, 128)
                .transpose(1, 0, 2).reshape(128, T * 128))
        in_maps.append({"G": np.ascontiguousarray(Gd), "S": np.ascontiguousarray(Sd)})

    # block-diagonal weights: BW[k][j*64+c, j*64+o] = W[o,c,k]
    BW = np.zeros((K, 128, 128), np.float32)
    wt = weight.transpose(2, 1, 0)                 # (k, c, o)
    BW[:, :64, :64] = wt
    BW[:, 64:, 64:] = wt
    BWp = np.ascontiguousarray(
        BW.transpose(1, 0, 2).reshape(128, K * 128)).astype(_G_NP)
    for m in in_maps:
        m["BW"] = BWp

    return in_maps, T_bk, T_blk, blk_base, (SLOTS, P)


def _build(T_bk, T_blk, blk_base, SLOTS):
    """Emit the Bass/Tile program (identical for all cores)."""
    if isinstance(SLOTS, tuple):
        SLOTS = SLOTS[0]
    f32 = mybir.dt.float32
    TT = SLOTS // 128

    nc = bass.Bass()
    G_d = nc.declare_dram_parameter("G", [128, TT * 256], _G_MY, isOutput=False)
    S_d = nc.declare_dram_parameter("S", [128, TT * 128], _S_MY, isOutput=False)
    BW_d = nc.declare_dram_parameter("BW", [128, K * 128], _G_MY, isOutput=False)
    Y_d = nc.declare_dram_parameter("Y", [NBLK, 128, 2 * 128], f32, isOutput=True)

    with TileContext(nc) as tc:
        with (
            tc.tile_pool(name="const", bufs=1) as cpool,
            tc.tile_pool(name="gp", bufs=5) as gpool,
            tc.tile_pool(name="sp", bufs=5) as spool,
            tc.tile_pool(name="zc", bufs=2) as zcpool,
            tc.tile_pool(name="ys", bufs=2) as yspool,
            tc.tile_pool(name="zp", bufs=5, space="PSUM") as zpool,
            tc.tile_pool(name="yp", bufs=2, space="PSUM") as ypool,
        ):
            bw = cpool.tile([128, K, 128], _G_MY)
            nc.scalar.dma_start(bw[:], BW_d[:])

            def emit_w(b, active, zc_t):
                # W contraction + writeback for block b (deferred one block so
                # a late zc never stalls the tensor queue ahead of the next
                # block's scatter matmuls)
                y_ps = ypool.tile([128, 256], f32, tag="y")
                for i, k in enumerate(active):
                    nc.tensor.matmul(
                        out=y_ps[:],
                        lhsT=bw[:, k, :],
                        rhs=zc_t[:, k, :],
                        start=(i == 0), stop=(i == len(active) - 1),
                    )
                y_sb = yspool.tile([128, 256], f32, tag="ysb")
                nc.scalar.copy(y_sb[:], y_ps[:])
                nc.sync.dma_start(Y_d[b], y_sb[:])

            pending = None
            for b in range(NBLK):
                T = int(T_blk[b])
                if T == 0:
                    continue
                tb0 = int(blk_base[b]) // 128
                g_t = gpool.tile([128, T, 256], _G_MY, tag="g")
                # split the big G stream across both HWDGE rings
                (nc.sync if b % 2 == 0 else nc.scalar).dma_start(
                    g_t[:], G_d[:, tb0 * 256:(tb0 + T) * 256])
                s_t = spool.tile([128, T, 128], _S_MY, tag="s")
                (nc.scalar if b % 2 == 0 else nc.sync).dma_start(
                    s_t[:], S_d[:, tb0 * 128:(tb0 + T) * 128])

                # PSUM accumulators: one bank per k-pair. start=True claims
                # the whole 2KB bank (zero region), so only the bank's FIRST
                # matmul starts and only its LAST stops; per-element
                # has_written turns the other first-touches into plain writes.
                z_tiles = [zpool.tile([128, 512], f32, tag="z", name=f"z{i}")
                           for i in range(5)]
                t_starts = np.concatenate([[0], np.cumsum(T_bk[b])[:-1]])
                for pair in range(5):
                    ks = [k for k in (2 * pair, 2 * pair + 1)
                          if k < K and T_bk[b][k] > 0]
                    mms = [(k, int(t_starts[k]) + ti, half)
                           for k in ks for ti in range(int(T_bk[b][k]))
                           for half in range(2)]
                    for i, (k, t, half) in enumerate(mms):
                        sub = k % 2
                        nc.tensor.matmul(
                            out=z_tiles[pair][:, sub * 256 + half * 128:
                                              sub * 256 + (half + 1) * 128],
                            lhsT=g_t[:, t, half * 128:(half + 1) * 128],
                            rhs=s_t[:, t, :],
                            start=(i == 0), stop=(i == len(mms) - 1),
                        )

                active = [k for k in range(K) if T_bk[b][k] > 0]
                zc_t = zcpool.tile([128, K, 256], _G_MY, tag="zc")
                for k in active:
                    pair, sub = k // 2, k % 2
                    nc.vector.tensor_copy(
                        out=zc_t[:, k, :],
                        in_=z_tiles[pair][:, sub * 256:(sub + 1) * 256])

                if pending is not None:
                    emit_w(*pending)
                pending = (b, active, zc_t)
            if pending is not None:
                emit_w(*pending)

    lower_extended_insts(nc)
    # this walrus build allows at most 1 sem-wait per instruction (2 on
    # event sems); split excess waits like Bacc does
    bass_rust.generate_event_semaphores(nc)
    return nc


def kernel(x, psi_idx, psi_vals, quadrature_weights, weight, bias):
    in_maps, T_bk, T_blk, blk_base, (SLOTS, P) = _prepare(
        x, psi_idx, psi_vals, quadrature_weights, weight
    )
    nc = _build(T_bk, T_blk, blk_base, SLOTS)
    core_ids = list(range(NCORES))
    res = run_bass_kernel_spmd(nc, in_maps, core_ids, trace=False)

    y = np.empty((B, COUT, N_OUT), np.float32)
    for c in core_ids:
        Yc = np.asarray(res.results[c]["Y"])          # (NBLK, 128, 256)
        # p = j*64+o, col = half*128+n, b = 2*half + j
        a = Yc.reshape(NBLK, 2, 64, 2, 128)           # (blk, j, o, half, n)
        a = a.transpose(3, 1, 2, 0, 4)                # (half, j, o, blk, n)
        y[:, :, c * PPC:(c + 1) * PPC] = a.reshape(B, COUT, PPC)
    y *= 2.0 ** (-P)
    y += bias.astype(np.float32)[None, :, None]
    return y


# revision 7
# speedup vs baseline: 1.0019x; 1.0019x over previous
"""DiscreteContinuousConv2d (sparse gnn-style conv) Trainium2 kernel.

Math: y[b,o,n] = bias[o] + sum_e psi[e] * qw[in_e] * sum_c W[o,c,k_e] * x[b, c, in_e]
      (edges e with out_e == n).

Strategy (8 NeuronCores, output sharded — no collectives):
  - Each core owns 2048 output points = 16 blocks of 128.
  - Host sorts edges by (core, block, k); pads each (block, k) group to a
    multiple of 128 ("tiles"), identical tile counts across cores (SPMD).
  - Host performs the per-edge gather (sparse im2col): G[slot, :] is the
    256-wide x row of the slot's in-point, pre-scaled by psi' =
    psi * qw[in] * 2^P (P keeps bf16 in range; y is unscaled on the host).
    G streams to the device as a dense (128, T*256) bf16 tile per block —
    the device never does random-access DMA.
  - With psi folded into G, the scatter matrices S[e, t, n] =
    (out_loc[slot] == n) are exact {0,1} one-hots — streamed as fp8
    (1.0 is exact) at half the bf16 footprint.
  - Per block on device:
      * scatter-add as matmul: z_pair[(row-half), (sub, half, n)] +=
        G_half.T @ S_tile, accumulated in PSUM (one bank per k-pair).
      * W contraction: y^T_half += BW_k.T @ z_k^T with BW_k the
        block-diagonal (over the 2 b's of a half) W_k^T. Accumulates in PSUM.
  - Host reassembles y from the per-core (block, p, n) outputs.
"""

import numpy as np
import ml_dtypes

import bass_rust
import concourse.bass as bass
from concourse import mybir
from concourse.bass_utils import run_bass_kernel_spmd
from concourse.library_overlay import lower_extended_insts
from concourse.tile import TileContext

B, CIN, COUT, K = 4, 64, 64, 9
N_IN = N_OUT = 16384
NCORES = 8
PPC = N_OUT // NCORES          # output points per core (2048)
NBLK = PPC // 128              # blocks per core (16)
ROW = B * CIN                  # gathered row width (256)

S_FP8 = True                   # S one-hot as fp8 (bf16 fallback)
_S_NP = ml_dtypes.float8_e4m3 if S_FP8 else ml_dtypes.bfloat16
_S_MY = mybir.dt.float8e4 if S_FP8 else mybir.dt.bfloat16
_G_NP = ml_dtypes.bfloat16
_G_MY = mybir.dt.bfloat16


def _prepare(x, psi_idx, psi_vals, quadrature_weights, weight):
    """Host-side sharding/sorting/layout. Returns per-core input maps."""
    xrow = np.ascontiguousarray(x.transpose(2, 0, 1).reshape(N_IN, ROW))

    k_idx = psi_idx[0].astype(np.int64)
    out_idx = psi_idx[1].astype(np.int64)
    in_idx = psi_idx[2].astype(np.int64)

    # fold quadrature into psi; rescale into dtype-friendly range (y is
    # divided by 2^P on the host afterwards)
    psi_q = psi_vals.astype(np.float64) * quadrature_weights.astype(np.float64)[in_idx]
    P = int(-np.ceil(np.log2(np.abs(psi_q).max())))
    psi_q = (psi_q * (2.0 ** P)).astype(np.float32)

    core = out_idx // PPC
    blk = (out_idx % PPC) // 128
    loc = out_idx % 128
    gid = (core * NBLK + blk) * K + k_idx          # group id, (core, blk, k)

    order = np.argsort(gid, kind="stable")
    gid_s = gid[order]
    in_s = in_idx[order]
    loc_s = loc[order]
    psi_s = psi_q[order]

    counts = np.bincount(gid_s, minlength=NCORES * NBLK * K).reshape(NCORES, NBLK, K)
    # tiles per (blk, k): shared across cores -> max
    T_bk = -(-counts.max(axis=0) // 128)           # (NBLK, K) ceil
    T_blk = T_bk.sum(axis=1)                       # (NBLK,)
    blk_base = np.concatenate([[0], np.cumsum(T_blk * 128)])  # slot offset per blk
    SLOTS = int(blk_base[-1])                      # total slots per core
    TT = SLOTS // 128

    # slot offset of each (blk, k) group
    k_base = np.zeros((NBLK, K), np.int64)
    for b in range(NBLK):
        k_base[b] = blk_base[b] + np.concatenate([[0], np.cumsum(T_bk[b] * 128)[:-1]])

    # destination slot for every (sorted) edge
    grp_start = np.zeros(NCORES * NBLK * K + 1, np.int64)
    np.cumsum(counts.reshape(-1), out=grp_start[1:])
    rank = np.arange(len(gid_s)) - grp_start[gid_s]
    g_core = gid_s // (NBLK * K)
    g_blk = (gid_s // K) % NBLK
    g_k = gid_s % K
    slot = k_base[g_blk, g_k] + rank               # slot within the core's stream

    in_maps = []
    for c in range(NCORES):
        m = g_core == c
        sl = slot[m]
        in_full = np.zeros(SLOTS, np.int64)
        in_full[sl] = in_s[m]
        psi_full = np.zeros(SLOTS, np.float32)
        psi_full[sl] = psi_s[m]
        # G: psi-scaled x rows, (SLOTS, 256) -> per blk (128, T*256)
        Gf = (xrow[in_full] * psi_full[:, None]).astype(_G_NP)
        Gd = np.empty((128, TT * 256), _G_NP)
        # S: {0,1} one-hot scatter tiles, (128, TT*128)
        Sf = np.zeros((SLOTS, 128), _S_NP)
        Sf[sl, loc_s[m]] = 1.0
        Sd = np.empty((128, TT * 128), _S_NP)
        for b in range(NBLK):
            s0, T = int(blk_base[b]), int(T_blk[b])
            t0 = s0 // 128
            Gd[:, t0 * 256:(t0 + T) * 256] = (
                Gf[s0:s0 + T * 128].reshape(T, 128, 256)
                .transpose(1, 0, 2).reshape(128, T * 256))
            Sd[:, t0 * 128:(t0 + T) * 128] = (
                Sf[s0:s0 + T * 128].reshape(T, 128, 128)
                .transpose(1, 0, 2).reshape(128, T * 128))
        in_maps.append({"G": np.ascontiguousarray(Gd), "S": np.ascontiguousarray(Sd)})

    # block-diagonal weights: BW[k][j*64+c, j*64+o] = W[o,c,k]
    BW = np.zeros((K, 128, 128), np.float32)
    wt = weight.transpose(2, 1, 0)                 # (k, c, o)
    BW[:, :64, :64] = wt
    BW[:, 64:, 64:] = wt
    BWp = np.ascontiguousarray(
        BW.transpose(1, 0, 2).reshape(128, K * 128)).astype(_G_NP)
    for m in in_maps:
        m["BW"] = BWp

    return in_maps, T_bk, T_blk, blk_base, (SLOTS, P)


def _build(T_bk, T_blk, blk_base, SLOTS):
    """Emit the Bass/Tile program (identical for all cores)."""
    if isinstance(SLOTS, tuple):
        SLOTS = SLOTS[0]
    f32 = mybir.dt.float32
    TT = SLOTS // 128

    nc = bass.Bass()
    G_d = nc.declare_dram_parameter("G", [128, TT * 256], _G_MY, isOutput=False)
    S_d = nc.declare_dram_parameter("S", [128, TT * 128], _S_MY, isOutput=False)
    BW_d = nc.declare_dram_parameter("BW", [128, K * 128], _G_MY, isOutput=False)
    Y_d = nc.declare_dram_parameter("Y", [NBLK, 128, 2 * 128], _G_MY, isOutput=True)

    with TileContext(nc) as tc:
        with (
            tc.tile_pool(name="const", bufs=1) as cpool,
            tc.tile_pool(name="gp", bufs=5) as gpool,
            tc.tile_pool(name="sp", bufs=5) as spool,
            tc.tile_pool(name="zc", bufs=2) as zcpool,
            tc.tile_pool(name="ys", bufs=2) as yspool,
            tc.tile_pool(name="zp", bufs=5, space="PSUM") as zpool,
            tc.tile_pool(name="yp", bufs=2, space="PSUM") as ypool,
        ):
            bw = cpool.tile([128, K, 128], _G_MY)
            nc.scalar.dma_start(bw[:], BW_d[:])

            def emit_w(b, active, zc_t):
                # W contraction + writeback for block b (deferred one block so
                # a late zc never stalls the tensor queue ahead of the next
                # block's scatter matmuls)
                y_ps = ypool.tile([128, 256], f32, tag="y")
                for i, k in enumerate(active):
                    nc.tensor.matmul(
                        out=y_ps[:],
                        lhsT=bw[:, k, :],
                        rhs=zc_t[:, k, :],
                        start=(i == 0), stop=(i == len(active) - 1),
                    )
                y_sb = yspool.tile([128, 256], _G_MY, tag="ysb")
                nc.scalar.copy(y_sb[:], y_ps[:])
                nc.sync.dma_start(Y_d[b], y_sb[:])

            pending = None
            for b in range(NBLK):
                T = int(T_blk[b])
                if T == 0:
                    continue
                tb0 = int(blk_base[b]) // 128
                g_t = gpool.tile([128, T, 256], _G_MY, tag="g")
                # G in chunks, alternating HWDGE rings: slice-level deps let
                # the scatter matmuls of a chunk start as soon as it lands
                GC = 9                 # tiles per chunk
                for j, gi in enumerate(range(0, T, GC)):
                    ge = min(T, gi + GC)
                    eng = nc.sync if (b + j) % 2 == 0 else nc.scalar
                    eng.dma_start(g_t[:, gi:ge, :],
                                  G_d[:, (tb0 + gi) * 256:(tb0 + ge) * 256])
                s_t = spool.tile([128, T, 128], _S_MY, tag="s")
                (nc.scalar if b % 2 == 0 else nc.sync).dma_start(
                    s_t[:], S_d[:, tb0 * 128:(tb0 + T) * 128])

                # PSUM accumulators: one bank per k-pair. start=True claims
                # the whole 2KB bank (zero region), so only the bank's FIRST
                # matmul starts and only its LAST stops; per-element
                # has_written turns the other first-touches into plain writes.
                z_tiles = [zpool.tile([128, 512], f32, tag="z", name=f"z{i}")
                           for i in range(5)]
                t_starts = np.concatenate([[0], np.cumsum(T_bk[b])[:-1]])
                for pair in range(5):
                    ks = [k for k in (2 * pair, 2 * pair + 1)
                          if k < K and T_bk[b][k] > 0]
                    mms = [(k, int(t_starts[k]) + ti, half)
                           for k in ks for ti in range(int(T_bk[b][k]))
                           for half in range(2)]
                    for i, (k, t, half) in enumerate(mms):
                        sub = k % 2
                        nc.tensor.matmul(
                            out=z_tiles[pair][:, sub * 256 + half * 128:
                                              sub * 256 + (half + 1) * 128],
                            lhsT=g_t[:, t, half * 128:(half + 1) * 128],
                            rhs=s_t[:, t, :],
                            start=(i == 0), stop=(i == len(mms) - 1),
                        )

                active = [k for k in range(K) if T_bk[b][k] > 0]
                zc_t = zcpool.tile([128, K, 256], _G_MY, tag="zc")
                for k in active:
                    pair, sub = k // 2, k % 2
                    nc.vector.tensor_copy(
                        out=zc_t[:, k, :],
                        in_=z_tiles[pair][:, sub * 256:(sub + 1) * 256])

                if pending is not None:
                    emit_w(*pending)
                pending = (b, active, zc_t)
            if pending is not None:
                emit_w(*pending)

    lower_extended_insts(nc)
    # this walrus build allows at most 1 sem-wait per instruction (2 on
    # event sems); split excess waits like Bacc does
    bass_rust.generate_event_semaphores(nc)
    return nc


def kernel(x, psi_idx, psi_vals, quadrature_weights, weight, bias):
    in_maps, T_bk, T_blk, blk_base, (SLOTS, P) = _prepare(
        x, psi_idx, psi_vals, quadrature_weights, weight
    )
    nc = _build(T_bk, T_blk, blk_base, SLOTS)
    core_ids = list(range(NCORES))
    res = run_bass_kernel_spmd(nc, in_maps, core_ids, trace=False)

    y = np.empty((B, COUT, N_OUT), np.float32)
    for c in core_ids:
        Yc = np.asarray(res.results[c]["Y"])          # (NBLK, 128, 256)
        # p = j*64+o, col = half*128+n, b = 2*half + j
        a = Yc.reshape(NBLK, 2, 64, 2, 128)           # (blk, j, o, half, n)
        a = a.transpose(3, 1, 2, 0, 4)                # (half, j, o, blk, n)
        y[:, :, c * PPC:(c + 1) * PPC] = a.reshape(B, COUT, PPC)
    y *= 2.0 ** (-P)
    y += bias.astype(np.float32)[None, :, None]
    return y
